# revision 18
# baseline (speedup 1.0000x reference)
"""Distributed GQA attention block (dense transformer) on 8 TRN2 NeuronCores.

Reference computation (per problem):
  xq = x @ wq.T ; xk = x @ wk.T ; xv = x @ wv.T      (torch-Linear style)
  RoPE (interleaved pairs) on xq, xk
  GQA causal attention (32 q heads, 8 kv heads, head_dim 128, seq 2048)
  out = attn_out @ wo.T

Sharding: tensor-parallel over heads. Core c gets q heads [4c, 4c+4) (rows
512c:512c+512 of wq) and kv head c. For the output projection, each core
AllGathers the (feature-major) attention output y of a token chunk from all
cores, then computes its own 512 OUTPUT columns of `out` with a local slice
of wo (rows 512c:512c+512 of wo); the host concatenates the column shards.

Device pipeline per core, built to keep the PE continuously busy (the PE
p-state only reaches 2.4 GHz after ~3us of gapless execution):
  Per 512-token chunk:
  1. Dense projection phase: k/v then 4 q blocks, weights stationary and
     xT moving so qT/kT land in [feature, token] layout. RoPE applied in
     [f, t] via rot(x) = x*CF + (Pm.T @ x)*SF. vT transposed to [t, dv].
  2. Attention in head PAIRS, jt-major: for each kv tile jt, scores for
     both heads share the kTt[:, jt] stationary; the AV matmuls run one
     jt-step BEHIND the scores so the exp (ACT engine) latency never
     stalls the in-order PE queue. wo matmuls of the PREVIOUS chunk are
     statically interleaved between jt steps as PE filler (the attention
     phase alone is ACT-bound).
  3. Softmax normalization (ones-matmul denominator, fast reciprocal,
     partition_broadcast, multiply) + y write + AllGather runs as a
     DEFERRED thunk: two jt-steps into the next group for interior
     chunks (so the in-order PE queue never waits on it), immediately at
     group end for chunks 0 and 3 (so their AllGathers fire earliest —
     chunk 3's trio piece flies while its last head computes). A dummy
     warmup AllGather in the preamble absorbs the first-collective cost.
  Head groups are 3+1: piece 0 (heads 0-2, 24 of 32 wo k-tiles) gathers
  well before piece 1, so the tail's wo runs mostly behind piece 1's
  AllGather. Engine/queue assignment: sync = x loads/prefetch + wq;
  scalar = exps + tables/wo preload; gpsimd = wkv preload, yt loads,
  y writes, AG triggers, broadcasts, out stores; DVE = rope, casts,
  exsum accumulation, tri masks, normalize, ow copies.
"""
import sys

sys.path.insert(0, "/opt/trn_rl_repo")

import numpy as np
import ml_dtypes

from concourse import bass, bacc, tile, mybir, bass_isa
from concourse.bass_utils import run_bass_kernel_spmd

N_CORES = 8
DIM = 4096
N_HEADS = 32
HEAD_DIM = 128
SEQ = 2048
ROPE_THETA = 10000.0

HQ = N_HEADS // N_CORES          # 4 local q heads
FQ = HQ * HEAD_DIM               # 512 q features per core
KT = DIM // 128                  # 32 contraction tiles
TT = SEQ // 128                  # 16 token tiles
NCH = 4                          # token chunks
CHUNK = SEQ // NCH               # 512
SCALE = 1.0 / float(np.sqrt(HEAD_DIM))

F32 = mybir.dt.float32
BF16 = mybir.dt.bfloat16
AL = mybir.AluOpType


def build_nc():
    nc = bacc.Bacc("TRN2", target_bir_lowering=False, debug=False,
                   num_devices=N_CORES)

    # ---- external inputs (host pre-casts to bf16, pre-transposes weights) --
    x_ext = nc.dram_tensor("xT", [DIM, SEQ], BF16, kind="ExternalInput")
    wqT_ext = nc.dram_tensor("wqT", [DIM, FQ], BF16, kind="ExternalInput")
    wkvT_ext = nc.dram_tensor("wkvT", [DIM, 256], BF16, kind="ExternalInput")
    woT_ext = nc.dram_tensor("woT", [DIM, FQ], BF16, kind="ExternalInput")
    cf_ext = nc.dram_tensor("cf", [128, SEQ], BF16, kind="ExternalInput")
    sf_ext = nc.dram_tensor("sf", [128, SEQ], BF16, kind="ExternalInput")
    pm_ext = nc.dram_tensor("pm", [128, 128], BF16, kind="ExternalInput")
    tri_ext = nc.dram_tensor("tri", [128, 128], BF16, kind="ExternalInput")
    id_ext = nc.dram_tensor("ident", [128, 128], BF16, kind="ExternalInput")

    out_ext = nc.dram_tensor("out", [SEQ, FQ], BF16, kind="ExternalOutput")

    warm_in = nc.dram_tensor("warm_in", [384, CHUNK], BF16)
    warm_out = nc.dram_tensor("warm_out", [3072, CHUNK], BF16,
                              addr_space="Shared")

    # ---- internal DRAM: per-chunk y head-pair pieces + their gathers ----
    y_dram = [[nc.dram_tensor(f"ych{c}p0", [384, CHUNK], BF16),
               nc.dram_tensor(f"ych{c}p1", [128, CHUNK], BF16)]
              for c in range(NCH)]
    ag_dram = [[nc.dram_tensor(f"agch{c}p0", [3072, CHUNK], BF16,
                               addr_space="Shared"),
                nc.dram_tensor(f"agch{c}p1", [1024, CHUNK], BF16,
                               addr_space="Shared")]
               for c in range(NCH)]

    with tile.TileContext(nc) as tc:
        pers_cm = tc.tile_pool(name="pers", bufs=1)
        pers = pers_cm.__enter__()
        wqT = pers.tile([128, KT, FQ], BF16, tag="wqT")      # [d, kt, f]
        woT = pers.tile([128, KT, FQ], BF16, tag="woT")      # [yf, kt, of]
        wkvT = pers.tile([128, KT, 256], BF16, tag="wkvT")   # [d, kt, kv|v]
        cf = pers.tile([128, SEQ], BF16, tag="cf")           # rope cos [f, t]
        sft = pers.tile([128, SEQ], BF16, tag="sf")          # rope sin [f, t]
        pm = pers.tile([128, 128], BF16, tag="pm")           # signed pairswap
        tri = pers.tile([128, 128], BF16, tag="tri")         # causal 128-blk
        ident = pers.tile([128, 128], BF16, tag="ident")
        kTt = pers.tile([128, SEQ], BF16, tag="kTt")         # [d, t]
        vS = pers.tile([128, TT, HEAD_DIM], BF16, tag="vS")  # [t_loc, tt, dv]
        ones_b = pers.tile([128, 1], BF16, tag="ones_b")

        with tc.tile_pool(name="ps_sc", bufs=2, space="PSUM") as psc, \
             tc.tile_pool(name="ps_o", bufs=3, space="PSUM") as po, \
             tc.tile_pool(name="ps_w", bufs=2, space="PSUM") as pw, \
             tc.tile_pool(name="ps_m", bufs=1, space="PSUM") as pmp, \
             tc.tile_pool(name="xp", bufs=9) as xp, \
             tc.tile_pool(name="qtp", bufs=1) as qtp, \
             tc.tile_pool(name="rp", bufs=2) as rp, \
             tc.tile_pool(name="sm", bufs=2) as smp, \
             tc.tile_pool(name="exp", bufs=6) as exp_, \
             tc.tile_pool(name="esp", bufs=4) as esp, \
             tc.tile_pool(name="yp", bufs=1) as yp, \
             tc.tile_pool(name="ytp", bufs=1) as ytp, \
             tc.tile_pool(name="owp", bufs=2) as owp:

            nc.any.memset(ones_b[:, :], 1.0)

            pending = []                 # deferred normalize+AG thunks

            def drain():
                while pending:
                    pending.pop(0)()

            def rope_block(src_sb, dst_ap, tslice):
                """dst = src*CF + (Pm.T @ src)*SF  on a [128, CHUNK] block."""
                ps_sw = psc.tile([128, CHUNK], F32, tag="sc")
                nc.tensor.matmul(ps_sw[:, :], pm[:, :], src_sb[:, :],
                                 start=True, stop=True)
                t1 = rp.tile([128, CHUNK], BF16, tag="t1")
                t2 = rp.tile([128, CHUNK], BF16, tag="t2")
                nc.vector.tensor_tensor(out=t1[:, :], in0=src_sb[:, :],
                                        in1=cf[:, tslice], op=AL.mult)
                nc.vector.tensor_tensor(out=t2[:, :], in0=ps_sw[:, :],
                                        in1=sft[:, tslice], op=AL.mult)
                nc.vector.tensor_tensor(out=dst_ap, in0=t1[:, :],
                                        in1=t2[:, :], op=AL.add)

            # ---------------- wo: loads + interleavable matmul thunks ------
            yts = {}

            def wo_load(c, pieces=(0, 1)):
                """Issue the gathered-y loads for chunk c (gpsimd queue).
                Piece 0 lands as soon as its (early) AllGather completes."""
                if c not in yts:
                    yts[c] = ytp.tile([128, KT, CHUNK], BF16, tag="yt",
                                      name="yt")
                yt = yts[c]
                for p in pieces:
                    nsub = 3 if p == 0 else 1
                    for s in range(nsub):
                        nc.gpsimd.dma_start(
                            out=yt[:, 24 * p + 8 * s:24 * p + 8 * (s + 1), :],
                            in_=ag_dram[c][p].ap()[1024 * s:1024 * (s + 1), :]
                            .rearrange("(kt p) t -> p kt t", p=128))

            def wo_fin(c, ps_w, tl):
                ow = owp.tile([128, CHUNK], BF16, tag="ow")
                nc.vector.tensor_copy(out=ow[:, :], in_=ps_w[:, :])
                r0 = CHUNK * c + 128 * tl
                nc.gpsimd.dma_start(out=out_ext[r0:r0 + 128, :],
                                    in_=ow[:, :])

            def wo_mms(c):
                """Thunk generator: wo matmuls for chunk c, token tiles in
                pairs, kt 0..31 per tile (both pieces long since arrived
                when interleaved into the next chunk's attention)."""
                yt = yts[c]
                for pair in ((0, 1), (2, 3)):
                    pws = [pw.tile([128, CHUNK], F32, tag="acc", name="wacc")
                           for _ in pair]
                    for lohi in ((0, 24), (24, KT)):
                        for ps_w, tl in zip(pws, pair):
                            for kt in range(*lohi):
                                def mk(ps_w=ps_w, tl=tl, kt=kt):
                                    nc.tensor.matmul(
                                        ps_w[:, :],
                                        yt[:, kt, 128 * tl:128 * (tl + 1)],
                                        woT[:, kt, :],
                                        start=(kt == 0),
                                        stop=(kt == KT - 1))
                                yield mk
                    for ps_w, tl in zip(pws, pair):
                        yield lambda ps_w=ps_w, tl=tl: wo_fin(c, ps_w, tl)

            def wo_tail(c):
                """Piece-major wo for the last chunk: all four token tiles'
                kt 0..15 (piece 0) first — they run while piece 1's
                AllGather is still in flight."""
                yt = yts[c]
                accs = [pw.tile([128, CHUNK], F32, tag="acc", name="wacc")
                        for _ in range(2)]
                accs += [po.tile([128, CHUNK], F32, tag="o", name="wacc2")
                         for _ in range(2)]
                for kt in range(KT):
                    for tl in range(4):
                        nc.tensor.matmul(
                            accs[tl][:, :],
                            yt[:, kt, 128 * tl:128 * (tl + 1)],
                            woT[:, kt, :],
                            start=(kt == 0), stop=(kt == KT - 1))

                for tl in range(4):
                    wo_fin(c, accs[tl], tl)

            for c in range(NCH):
                tsl = slice(CHUNK * c, CHUNK * (c + 1))
                # ---- preamble loads (c == 0) ----
                if c == 0:
                    xgs = []
                    for g in range(8):
                        nc.gpsimd.dma_start(
                            out=wkvT[:, 4 * g:4 * (g + 1), :],
                            in_=wkvT_ext[512 * g:512 * (g + 1), :]
                            .rearrange("(kt p) f -> p kt f", p=128))
                        xg = xp.tile([128, 4, CHUNK], BF16, tag="xT")
                        nc.sync.dma_start(
                            out=xg[:, :, :],
                            in_=x_ext[512 * g:512 * (g + 1), tsl]
                            .rearrange("(kt p) t -> p kt t", p=128))
                        xgs.append(xg)
                    nc.scalar.dma_start(out=pm[:, :], in_=pm_ext[:, :])
                    nc.scalar.dma_start(out=tri[:, :], in_=tri_ext[:, :])
                    nc.scalar.dma_start(out=ident[:, :], in_=id_ext[:, :])
                    nc.scalar.dma_start(out=cf[:, :], in_=cf_ext[:, :])
                    nc.scalar.dma_start(out=sft[:, :], in_=sf_ext[:, :])
                    for g in range(4):
                        nc.sync.dma_start(
                            out=wqT[:, 8 * g:8 * (g + 1), :],
                            in_=wqT_ext[1024 * g:1024 * (g + 1), :]
                            .rearrange("(kt p) f -> p kt f", p=128))
                    for g in range(4):
                        nc.scalar.dma_start(
                            out=woT[:, 8 * g:8 * (g + 1), :],
                            in_=woT_ext[1024 * g:1024 * (g + 1), :]
                            .rearrange("(kt p) f -> p kt f", p=128))
                    nc.gpsimd.collective_compute(
                        "AllGather", AL.bypass,
                        replica_groups=[list(range(N_CORES))],
                        ins=[warm_in.ap().opt()],
                        outs=[warm_out.ap().opt()])
                else:
                    xgs = xgs_next  # prefetched during chunk c-1  # noqa: F821

                def xt(k):
                    return xgs[k // 4][:, k % 4, :]

                # ---- projection phase ----
                # fire the previous pair's AG first (pure DVE/gpsimd), then
                # issue the gathered-y loads it unblocks.
                drain()
                if c >= 1:
                    wo_load(c - 1)
                ps_k = pw.tile([128, CHUNK], F32, tag="acc")
                ps_v = pw.tile([128, CHUNK], F32, tag="acc")
                for k in range(KT):
                    nc.tensor.matmul(ps_k[:, :], wkvT[:, k, 0:128],
                                     xt(k),
                                     start=(k == 0), stop=(k == KT - 1))
                    nc.tensor.matmul(ps_v[:, :], wkvT[:, k, 128:256],
                                     xt(k),
                                     start=(k == 0), stop=(k == KT - 1))
                kb = rp.tile([128, CHUNK], BF16, tag="qb")
                nc.vector.tensor_copy(out=kb[:, :], in_=ps_k[:, :])
                rope_block(kb, kTt[:, tsl], tsl)
                # v -> vT [dv, t], then transpose to natural [t, dv]
                vtb = rp.tile([128, CHUNK], BF16, tag="qb")
                nc.vector.tensor_copy(out=vtb[:, :], in_=ps_v[:, :])
                for tl in range(4):
                    ps_tr = pmp.tile([128, 128], BF16, tag="m")
                    nc.tensor.transpose(ps_tr[:, :],
                                        vtb[:, 128 * tl:128 * (tl + 1)],
                                        ident[:, :])
                    nc.vector.tensor_copy(out=vS[:, 4 * c + tl, :],
                                          in_=ps_tr[:, :])
                qT = qtp.tile([128, HQ, CHUNK], BF16, tag="qT")
                for b in range(HQ):
                    ps_q = pw.tile([128, CHUNK], F32, tag="acc")
                    for k in range(KT):
                        nc.tensor.matmul(ps_q[:, :],
                                         wqT[:, k, 128 * b:128 * (b + 1)],
                                         xt(k),
                                         start=(k == 0),
                                         stop=(k == KT - 1))
                    qb = rp.tile([128, CHUNK], BF16, tag="qb")
                    nc.vector.tensor_copy(out=qb[:, :], in_=ps_q[:, :])
                    rope_block(qb, qT[:, b, :], tsl)

                # ---- x prefetch for chunk c+1 (sync queue) ----
                if c + 1 < NCH:
                    nsl = slice(CHUNK * (c + 1), CHUNK * (c + 2))
                    xgs_next = []
                    for g in range(8):
                        xg = xp.tile([128, 4, CHUNK], BF16, tag="xT")
                        nc.sync.dma_start(
                            out=xg[:, :, :],
                            in_=x_ext[512 * g:512 * (g + 1), nsl]
                            .rearrange("(kt p) t -> p kt t", p=128))
                        xgs_next.append(xg)

                # ---- attention: head pairs, jt-major, AV one step behind;
                # wo(c-1) matmuls as PE filler ----
                njt = 4 * (c + 1)
                filler = wo_mms(c - 1) if c >= 1 else iter(())
                n_fill = 34 * HQ
                per_step = -(-n_fill // (2 * njt)) if c >= 1 else 0

                def fill(n):
                    for _ in range(n):
                        f = next(filler, None)
                        if f is None:
                            return
                        f()

                def normalize_thunk(c, pr, ps_os, exsum, grp):
                    def thunk():
                        y_sb = yp.tile([128, len(grp), CHUNK], BF16,
                                       tag=f"y{len(grp)}", name="y_sb")
                        for h in grp:
                            esb = rp.tile([128, CHUNK], BF16, tag="esb",
                                          name="esb")
                            nc.vector.tensor_copy(out=esb[:, :],
                                                  in_=exsum[h][:, :])
                            ps_l = pmp.tile([1, CHUNK], F32, tag="m",
                                            name="ps_l")
                            nc.tensor.matmul(ps_l[:, :], ones_b[:, :],
                                             esb[:, :], start=True,
                                             stop=True)
                            rr = smp.tile([1, CHUNK], F32, tag="rr")
                            nc.vector.reciprocal_approx_fast(out=rr[:, :],
                                                             in_=ps_l[:, :])
                            rrb = smp.tile([1, CHUNK], BF16, tag="rrb")
                            nc.vector.tensor_copy(out=rrb[:, :],
                                                  in_=rr[:, :])
                            bc = rp.tile([128, CHUNK], BF16, tag="bc")
                            nc.gpsimd.partition_broadcast(bc[:, :],
                                                          rrb[:, :])
                            nc.vector.tensor_tensor(
                                out=y_sb[:, h - grp[0], :],
                                in0=ps_os[h][:, :], in1=bc[:, :],
                                op=AL.mult)
                        nc.gpsimd.dma_start(
                            out=y_dram[c][pr].ap()
                            .rearrange("(hh p) t -> p hh t", p=128),
                            in_=y_sb[:, :, :])
                        nc.gpsimd.collective_compute(
                            "AllGather", AL.bypass,
                            replica_groups=[list(range(N_CORES))],
                            ins=[y_dram[c][pr].ap().opt()],
                            outs=[ag_dram[c][pr].ap().opt()])
                    return thunk

                for pr, grp in enumerate(((0, 1, 2), (3,))):
                    ps_os = {}
                    exsum = {}
                    for h in grp:
                        ps_os[h] = po.tile([128, CHUNK], F32, tag="o",
                                           name=f"ps_o{h}")
                        exsum[h] = esp.tile([128, CHUNK], F32, tag="es",
                                            name=f"exsum{h}")
                    prev = None  # (jt, lo, {h: exf})
                    for jt in range(njt):
                        p = jt - 4 * c
                        lo = 128 * p if p > 0 else 0
                        exfs = {}
                        ps_ss = {}
                        for h in grp:
                            ps_s = psc.tile([128, CHUNK], F32, tag="sc",
                                            name="ps_s")
                            nc.tensor.matmul(
                                ps_s[:, lo:CHUNK],
                                kTt[:, 128 * jt:128 * (jt + 1)],
                                qT[:, h, lo:CHUNK],
                                start=True, stop=True)
                            ps_ss[h] = ps_s
                        for h in grp:
                            exf = exp_.tile([128, CHUNK], BF16, tag="ex",
                                            name="exf")
                            nc.scalar.activation(
                                out=exf[:, lo:CHUNK],
                                in_=ps_ss[h][:, lo:CHUNK],
                                func=mybir.ActivationFunctionType.Exp,
                                scale=SCALE)
                            if p >= 0:
                                nc.vector.tensor_tensor(
                                    out=exf[:, lo:lo + 128],
                                    in0=exf[:, lo:lo + 128],
                                    in1=tri[:, :], op=AL.mult)
                            exfs[h] = exf
                        if prev is not None:
                            pjt, plo, pexfs = prev
                            for h in grp:
                                nc.tensor.matmul(
                                    ps_os[h][:, plo:CHUNK],
                                    vS[:, pjt, :],
                                    pexfs[h][:, plo:CHUNK],
                                    start=(pjt == 0), stop=False)
                        for h in grp:
                            if jt == 0:
                                nc.vector.tensor_copy(out=exsum[h][:, :],
                                                      in_=exfs[h][:, :])
                            else:
                                nc.vector.tensor_tensor(
                                    out=exsum[h][:, lo:CHUNK],
                                    in0=exsum[h][:, lo:CHUNK],
                                    in1=exfs[h][:, lo:CHUNK], op=AL.add)
                        if jt == 2:
                            drain()   # previous pair's normalize + AG
                        fill(per_step)
                        prev = (jt, lo, exfs)
                    # flush last AV
                    pjt, plo, pexfs = prev
                    for h in grp:
                        nc.tensor.matmul(ps_os[h][:, plo:CHUNK],
                                         vS[:, pjt, :],
                                         pexfs[h][:, plo:CHUNK],
                                         start=(pjt == 0), stop=True)
                    pending.append(
                        normalize_thunk(c, pr, ps_os, exsum, grp))
                    if c == 0 or c == NCH - 1:
                        drain()
                    if c == NCH - 1 and pr == 0:
                        wo_load(c, pieces=(0,))
                # drain any remaining wo filler of chunk c-1
                fill(n_fill)
            # ---- tail: chunk 3 pair 1 normalize + AG, then piece-major wo
            drain()
            wo_load(NCH - 1, pieces=(1,))
            wo_tail(NCH - 1)

        pers_cm.__exit__(None, None, None)

    nc.finalize()
    return nc


_NC_CACHE = None


def _get_nc():
    global _NC_CACHE
    if _NC_CACHE is None:
        _NC_CACHE = build_nc()
    return _NC_CACHE


def _host_constants():
    bf = ml_dtypes.bfloat16
    m = np.arange(64, dtype=np.float64)
    freqs = 1.0 / (ROPE_THETA ** (2.0 * m / HEAD_DIM))
    t = np.arange(SEQ, dtype=np.float64)
    ang = np.outer(freqs, t)                                 # [64, SEQ]
    cfv = np.repeat(np.cos(ang), 2, axis=0).astype(bf)       # [128, SEQ]
    sfv = np.repeat(np.sin(ang), 2, axis=0).astype(bf)
    # signed pair swap: out[2m] = -in[2m+1], out[2m+1] = +in[2m]
    # out = Pm.T @ in  ->  Pm[2m+1, 2m] = -1 ; Pm[2m, 2m+1] = +1
    pmv = np.zeros((128, 128), np.float32)
    idx = np.arange(0, 128, 2)
    pmv[idx + 1, idx] = -1.0
    pmv[idx, idx + 1] = 1.0
    pmv = pmv.astype(bf)
    j = np.arange(128)[:, None]
    i = np.arange(128)[None, :]
    triv = (j <= i).astype(np.float32).astype(bf)
    identv = np.eye(128, dtype=bf)
    return cfv, sfv, pmv, triv, identv


def _make_in_maps(x, wq, wk, wv, wo):
    cfv, sfv, pmv, triv, identv = _host_constants()
    bf = ml_dtypes.bfloat16
    xT2 = np.ascontiguousarray(x.reshape(SEQ, DIM).astype(bf).T)
    wqT = np.ascontiguousarray(wq.T.astype(bf))              # [DIM, 4096]
    wkT = wk.T.astype(bf)                                    # [DIM, 1024]
    wvT = wv.T.astype(bf)
    in_maps = []
    for c in range(N_CORES):
        wkvT = np.concatenate([wkT[:, HEAD_DIM * c:HEAD_DIM * (c + 1)],
                               wvT[:, HEAD_DIM * c:HEAD_DIM * (c + 1)]],
                              axis=1)
        # rows of wo for OUR output columns, transposed: [DIM(yfeat), FQ].
        # The AllGather pieces deliver y-features in order
        # [rank0 h012, rank1 h012, ..., rank7 h012, rank0 h3, ...]:
        # reorder woT rows to match.
        woTc = wo[FQ * c:FQ * (c + 1), :].T.astype(bf)       # [DIM, FQ]
        w3 = woTc.reshape(N_CORES, FQ, FQ)
        woTc = np.ascontiguousarray(np.concatenate(
            [w3[:, 0:384, :].reshape(3072, FQ),
             w3[:, 384:512, :].reshape(1024, FQ)], axis=0))
        in_maps.append({
            "xT": xT2,
            "wqT": np.ascontiguousarray(wqT[:, FQ * c:FQ * (c + 1)]),
            "wkvT": np.ascontiguousarray(wkvT),
            "woT": woTc,
            "cf": cfv, "sf": sfv, "pm": pmv, "tri": triv, "ident": identv,
        })
    return in_maps


def _assemble(results):
    # core c holds out[:, 512c:512c+512]
    cols = [np.asarray(results[c]["out"]).astype(np.float32)
            for c in range(N_CORES)]
    return np.concatenate(cols, axis=1).reshape(1, SEQ, DIM)


def run(inputs, trace=False, tmpdir=None):
    nc = _get_nc()
    in_maps = _make_in_maps(inputs["x"], inputs["wq"], inputs["wk"],
                            inputs["wv"], inputs["wo"])
    res = run_bass_kernel_spmd(nc, in_maps, list(range(N_CORES)),
                               trace=trace, tmpdir=tmpdir)
    return _assemble(res.results), res


def kernel(x, start_pos, wq, wk, wv, wo):
    out, _ = run({"x": np.asarray(x), "wq": np.asarray(wq),
                  "wk": np.asarray(wk), "wv": np.asarray(wv),
                  "wo": np.asarray(wo)})
    return out


if __name__ == "__main__":
    rng = np.random.default_rng(0)
    x = rng.standard_normal((1, 2048, 4096)).astype(np.float32)
    wq = (rng.standard_normal((DIM, DIM)) * DIM ** -0.5).astype(np.float32)
    wk = (rng.standard_normal((1024, DIM)) * DIM ** -0.5).astype(np.float32)
    wv = (rng.standard_normal((1024, DIM)) * DIM ** -0.5).astype(np.float32)
    wo = (rng.standard_normal((DIM, DIM)) * DIM ** -0.5).astype(np.float32)
    out = kernel(x, 0, wq, wk, wv, wo)
    print(out.shape, out.dtype, np.abs(out).mean())
